# revision 47
# baseline (speedup 1.0000x reference)
"""BEiT-style windowed attention (B=32, N=577, D=768, 12 heads) on 8 TRN2 cores.

Data-parallel over batch (4 elements/core, no collectives).

qkv projection runs in fp8-e4m3 DoubleRow mode (0.5 PE cycles/row) with an
error-compensated hi+lo split: x = (x8h + x8l)/SX and W = (w8h + w8l)/SW,
computing x@W as the hh + hl + lh DoubleRow terms (the lo*lo term is below
fp16 noise). That is 4.5 effective f16 contraction steps instead of 6 at
near-fp16 accuracy. The SX*SW descale folds into the exp() scale operand
(scores path) and the rowsum ones-column value (v path), so no extra
elementwise work is needed. Scores/P@V/proj stay fp16; fp32 PSUM throughout.

Per-core dataflow (per batch element):
  qkT  [d,tok] = W8(T) @ x8              (fp8 DR hi/lo, 9 matmuls per tile)
  v    [tok,d] = x8(T) @ W8              (+const col = SX*SW for rowsum)
  S.T  [k,q]   = kT(T) @ qT              (fp16, K=64)
  P    = exp(S.T * SCALE/(SX*SW)^2) * exp(relbias.T)
  O_un [q,d+1] = P(T) @ v_aug            (col 64 = SX*SW * rowsum)
  O    = O_un[:, :64] * recip(rowsum)    (scales cancel exactly)
  OT   = dma_transpose(O)                (DMA xbar, keeps PE/Act free)
  out  [tok,od] = OT(T) @ W_projT + b

The PE instruction stream is software-pipelined: P@V of head h-2 is emitted
after scores of head h, and qkv tiles of batch b+1 / output-projection tiles
of batch b-1 are spread between score tiles as fillers so the in-order PE
queue always has ready work while exp (Act) and bias-mul (GpSimd) of earlier
score tiles complete. PSUM accumulation groups are started/stopped once per
2KB zero region (bank), with sub-chunks accumulating into the started group.
"""

import numpy as np

import concourse.bass as bass
import concourse.tile as tile
from concourse import bacc
from concourse import mybir
from concourse.bass_utils import run_bass_kernel_spmd

B, N, D = 32, 577, 768
NH, DH = 12, 64
NCORES = 8
BL = B // NCORES            # 4 batch elements per core
SCALE = DH ** -0.5
SX, SW = 4.0, 64.0            # fp8 hi/lo scales for x and W_qkv
NP = 608                      # x8 padded row length (4B-aligned subrows)
SO, SWP = 64.0, 64.0          # fp8 hi/lo scales for O and W_proj
KT = D // 128               # 6 contraction tiles over D
TT = (N + 127) // 128       # 5 token tiles (4x128 + 65)
BF16 = np.float16

F32 = mybir.dt.float32
BF = mybir.dt.float16
F8 = mybir.dt.float8e4
DR = mybir.MatmulPerfMode.DoubleRow


def tok_m(t):
    return min(128, N - 128 * t)


def _build_nc():
    nc = bacc.Bacc()

    x8h_d = nc.declare_dram_parameter("x8h", [BL, 128, KT, NP], F8, isOutput=False)
    x8l_d = nc.declare_dram_parameter("x8l", [BL, 128, KT, NP], F8, isOutput=False)
    w8h_d = nc.declare_dram_parameter("w8h", [128, KT, 3 * D], F8, isOutput=False)
    w8l_d = nc.declare_dram_parameter("w8l", [128, KT, 3 * D], F8, isOutput=False)
    wp8h_d = nc.declare_dram_parameter("wp8h", [128, KT, D], F8, isOutput=False)
    wp8l_d = nc.declare_dram_parameter("wp8l", [128, KT, D], F8, isOutput=False)
    biasT_d = nc.declare_dram_parameter("biasT", [128, NH, TT, N], BF, isOutput=False)
    qkvb_d = nc.declare_dram_parameter("qkvb", [128, 18], F32, isOutput=False)
    vb_d = nc.declare_dram_parameter("vb", [1, D], BF, isOutput=False)
    pb_d = nc.declare_dram_parameter("pb", [1, D], BF, isOutput=False)
    out_d = nc.declare_dram_parameter("out", [BL, N, D], BF, isOutput=True)

    Exp = mybir.ActivationFunctionType.Exp
    MT_ORDER = [t for i in range(KT) for t in (i, KT + i)]
    QCH = [(0, 512), (512, N - 512)]          # free-dim chunks over 577
    DCH = [(0, 512), (512, D - 512)]          # free-dim chunks over 768
    QCH8 = [(0, 256), (256, 256), (512, N - 512)]    # DR moving <= 2*256
    DCH8 = [(0, 256), (256, 256), (512, 256)]

    with tile.TileContext(nc) as tc:
        with (
            tc.tile_pool(name="singles", bufs=1) as singles,
            tc.tile_pool(name="xt", bufs=2) as xt_pool,
            tc.tile_pool(name="qkt", bufs=2) as qkt_pool,
            tc.tile_pool(name="vbuf", bufs=2) as v_pool,
            tc.tile_pool(name="exps", bufs=15) as exps_pool,
            tc.tile_pool(name="obuf", bufs=1) as o_pool,
            tc.tile_pool(name="otb", bufs=1) as ot_pool,
            tc.tile_pool(name="outs", bufs=2) as out_pool,
            tc.tile_pool(name="small", bufs=2) as small_pool,
            tc.tile_pool(name="ps512", bufs=2, space="PSUM") as ps512,
            tc.tile_pool(name="ps128", bufs=2, space="PSUM") as ps128,
            tc.tile_pool(name="psS", bufs=2, space="PSUM") as psS_pool,
        ):
            state = {}

            def make_qkv_units(b, split_x=False):
                """Per-tile qkv closures for batch b (used as PE fillers).

                fp8 hi/lo DoubleRow: psum accumulates 9 DR matmuls per tile
                (3 kt-pairs x {hh, hl, lh}), worth 4.5 f16 contraction steps
                instead of 6. Result is SX*SW-scaled; the scale cancels in
                the exp (scores) and the rowsum ones-column (P@V).
                """
                x8h = xt_pool.tile([128, KT, NP], F8, name="x8h", tag="x8h")
                x8l = xt_pool.tile([128, KT, NP], F8, name="x8l", tag="x8l")
                if split_x:
                    state["x0"] = (x8h, x8l)
                    nc.sync.dma_start(out=x8h[:, 0:2, :], in_=x8h_d[b, :, 0:2, :])
                    nc.sync.dma_start(out=x8l[:, 0:2, :], in_=x8l_d[b, :, 0:2, :])
                else:
                    nc.sync.dma_start(out=x8h, in_=x8h_d[b])
                    nc.sync.dma_start(out=x8l, in_=x8l_d[b])
                qkT = qkt_pool.tile([128, 2 * KT, N], BF, name="qkT", tag="qkT")
                v_sb = v_pool.tile([128, TT, NH * 65], BF, name="v", tag="v")
                v_str = v_sb.rearrange("p t (h c) -> p t h c", c=65)
                state[b] = (qkT, v_sb)
                TERMS = ((0, 0), (0, 1), (1, 0))     # (x hi/lo, w hi/lo)

                def qk_unit(mt):
                    pss = [ps512.tile([128, 512], F32, name="ps_qk0", tag="a"),
                           ps128.tile([128, 128], F32, name="ps_qk1", tag="b")]
                    n = 0
                    for kp in range(KT // 2):
                        for xi, wi in TERMS:
                            n += 1
                            xa = x8l if xi else x8h
                            wa = w8l if wi else w8h
                            for ci, (c0, w) in enumerate(QCH8):
                                nc.tensor.matmul(
                                    pss[0][:, c0:c0 + w] if ci < 2
                                    else pss[1][:, :w],
                                    wa[:, 2 * kp:2 * kp + 2,
                                       128 * mt:128 * (mt + 1)],
                                    xa[:, 2 * kp:2 * kp + 2, c0:c0 + w],
                                    start=(n == 1 and ci in (0, 2)),
                                    stop=(n == 9 and ci in (1, 2)),
                                    perf_mode=DR,
                                )
                    for ci, (c0, w) in enumerate(QCH):
                        nc.vector.tensor_add(
                            qkT[:, mt, c0:c0 + w], pss[ci][:, :w],
                            qkvb[:, mt:mt + 1].to_broadcast([128, w]),
                        )

                def v_unit(tt):
                    if tt == 0:
                        nc.vector.memset(v_str[:, :, :, 64:65], SX * SW / SO)
                    m = tok_m(tt)
                    pss = [ps512.tile([128, 512], F32, name="ps_v0", tag="a"),
                           ps512.tile([128, 512], F32, name="ps_v1", tag="a")]
                    n = 0
                    for kp in range(KT // 2):
                        for xi, wi in TERMS:
                            n += 1
                            xa = x8l if xi else x8h
                            wa = w8l if wi else w8h
                            for ci, (c0, w) in enumerate(DCH8):
                                nc.tensor.matmul(
                                    pss[0][:m, c0:c0 + w] if c0 < 512
                                    else pss[1][:m, c0 - 512:c0 - 512 + w],
                                    xa[:, 2 * kp:2 * kp + 2,
                                       128 * tt:128 * tt + m],
                                    wa[:, 2 * kp:2 * kp + 2,
                                       2 * D + c0:2 * D + c0 + w],
                                    start=(n == 1 and ci in (0, 2)),
                                    stop=(n == 9 and ci in (1, 2)),
                                    perf_mode=DR,
                                )
                    for ci, (c0, w) in enumerate(DCH):
                        nh0, nh1 = c0 // 64, (c0 + w) // 64
                        src_ps = (pss[ci][:m, :w] if ci == 0
                                  else pss[1][:m, :w])
                        nc.vector.tensor_add(
                            v_str[:m, tt, nh0:nh1, 0:64],
                            src_ps.rearrange("p (h c) -> p h c", c=64),
                            vbias[:m, c0:c0 + w].rearrange("p (h c) -> p h c", c=64),
                        )

                qk_units = [lambda mt=mt: qk_unit(mt) for mt in MT_ORDER]
                v_units = [lambda tt=tt: v_unit(tt) for tt in range(TT)]
                return {"early": qk_units[0:6], "later": qk_units[6:12],
                        "v": v_units}

            def emit_scores(b, h, fill):
                qkT, _ = state[b]
                qT = qkT[64 * (h % 2):64 * (h % 2) + 64, h // 2, :]
                kTh = qkT[64 * (h % 2):64 * (h % 2) + 64, KT + h // 2, :]
                expS = [exps_pool.tile([128, N], BF, name="expS", tag="es")
                        for _ in range(TT)]
                for kt in range(TT):
                    km = tok_m(kt)
                    ps_s = psS_pool.tile([128, N], F32, name="ps_s")
                    for ci, (c0, w) in enumerate(QCH):
                        nc.tensor.matmul(
                            ps_s[:km, c0:c0 + w],
                            kTh[:, 128 * kt:128 * kt + km],
                            qT[:, c0:c0 + w],
                            start=True, stop=True,
                        )
                    nc.scalar.activation(expS[kt][:km, :], ps_s[:km, :], Exp,
                                         scale=SCALE / (SX * SW) ** 2)
                    # exp(rel_bias) multiply, host-precomputed; mostly on the
                    # otherwise idle GpSimd
                    nc.gpsimd.tensor_mul(
                        expS[kt][:km, :], expS[kt][:km, :],
                        biasT[:km, h, kt, :],
                    )
                    fill()
                return expS

            def emit_pav(b, h, expS):
                _, v_sb = state[b]
                o_sb = state[b, "o"]
                for qt in range(TT):
                    qm = tok_m(qt)
                    ps_o = ps128.tile([128, 128], F32, name="ps_o", tag="b")
                    for kt in range(TT):
                        km = tok_m(kt)
                        nc.tensor.matmul(
                            ps_o[:qm, :65],
                            expS[kt][:km, 128 * qt:128 * qt + qm],
                            v_sb[:km, kt, 65 * h:65 * h + 65],
                            start=(kt == 0), stop=(kt == TT - 1),
                        )
                    rcp = small_pool.tile([128, 1], F32, name="rcp", tag="rcp")
                    nc.vector.reciprocal(rcp[:qm], ps_o[:qm, 64:65])
                    nc.vector.tensor_mul(
                        o_sb[:qm, qt, 64 * h:64 * h + 64],
                        ps_o[:qm, 0:64],
                        rcp[:qm, 0:1].to_broadcast([qm, 64]),
                    )

            def make_proj_units(b):
                """DMA transposes + hi/lo split + fp8-DR output projection.

                o_sb holds SO*O (via the ones column); the transposed oT f16
                is split per token-tile into fp8 hi+lo on GpSimd, then the
                projection runs as 9 DoubleRow matmuls per tile. The
                SO*SWP descale fuses into the scalar_tensor_tensor copy.
                """
                o_sb = state[b, "o"]
                oT = ot_pool.tile([128, KT, TT, 128], BF, name="oT", tag="oT")
                oT8h = ot_pool.tile([128, KT, TT, 128], F8, name="oT8h",
                                    tag="oT8h")
                oT8l = ot_pool.tile([128, KT, TT, 128], F8, name="oT8l",
                                    tag="oT8l")
                for qt in range(TT):
                    nc.sync.dma_start_transpose(
                        oT[:, :, qt, :], o_sb[:, qt, :])

                def split(tt):
                    nc.gpsimd.tensor_copy(oT8h[:, :, tt, :], oT[:, :, tt, :])
                    nc.gpsimd.tensor_sub(
                        oT8l[:, :, tt, :], oT[:, :, tt, :], oT8h[:, :, tt, :])
                split(0)
                split(1)

                def proj_unit(tt):
                    m = tok_m(tt)
                    if tt + 2 < TT:
                        split(tt + 2)
                    out_sb = out_pool.tile([128, D], BF, name="out", tag="out")
                    pss = [ps512.tile([128, 512], F32, name="ps_p0", tag="a"),
                           ps512.tile([128, 512], F32, name="ps_p1", tag="a")]
                    n = 0
                    for kp in range(KT // 2):
                        for oa, wa in ((oT8h, wp8h), (oT8h, wp8l),
                                       (oT8l, wp8h)):
                            n += 1
                            for ci, (c0, w) in enumerate(DCH8):
                                nc.tensor.matmul(
                                    pss[0][:m, c0:c0 + w] if c0 < 512
                                    else pss[1][:m, c0 - 512:c0 - 512 + w],
                                    oa[:, 2 * kp:2 * kp + 2, tt, :m],
                                    wa[:, 2 * kp:2 * kp + 2, c0:c0 + w],
                                    start=(n == 1 and ci in (0, 2)),
                                    stop=(n == 9 and ci in (1, 2)),
                                    perf_mode=DR,
                                )
                    for ci, (c0, w) in enumerate(DCH):
                        src_ps = pss[ci][:m, :w] if ci == 0 else pss[1][:m, :w]
                        nc.vector.scalar_tensor_tensor(
                            out_sb[:m, c0:c0 + w], src_ps,
                            1.0 / (SO * SWP), pbias[:m, c0:c0 + w],
                            op0=mybir.AluOpType.mult,
                            op1=mybir.AluOpType.add,
                        )
                    nc.sync.dma_start(
                        out=out_d[b, 128 * tt:128 * tt + m, :],
                        in_=out_sb[:m, :],
                    )

                return [lambda tt=tt: proj_unit(tt) for tt in range(TT)]

            # ---- one-time loads, ordered so the first qk matmuls can
            # start as early as possible (q rows of W, then x, then k/v) ----
            qkvb = singles.tile([128, 18], F32)
            w8h = singles.tile([128, KT, 3 * D], F8)
            w8l = singles.tile([128, KT, 3 * D], F8)
            first_units = None
            for kp in range(KT // 2):
                k0 = 2 * kp
                nc.sync.dma_start(out=w8h[:, k0:k0 + 2, 0:D],
                                  in_=w8h_d[:, k0:k0 + 2, 0:D])
                nc.sync.dma_start(out=w8l[:, k0:k0 + 2, 0:D],
                                  in_=w8l_d[:, k0:k0 + 2, 0:D])
                if first_units is None:
                    first_units = make_qkv_units(0, split_x=True)
                else:
                    if k0 == 2:
                        nc.sync.dma_start(out=qkvb, in_=qkvb_d[:])
                    x8h0, x8l0 = state["x0"]
                    nc.sync.dma_start(out=x8h0[:, k0:k0 + 2, :],
                                      in_=x8h_d[0, :, k0:k0 + 2, :])
                    nc.sync.dma_start(out=x8l0[:, k0:k0 + 2, :],
                                      in_=x8l_d[0, :, k0:k0 + 2, :])
            nc.sync.dma_start(out=w8h[:, :, D:2 * D], in_=w8h_d[:, :, D:2 * D])
            nc.sync.dma_start(out=w8l[:, :, D:2 * D], in_=w8l_d[:, :, D:2 * D])
            nc.sync.dma_start(out=w8h[:, :, 2 * D:], in_=w8h_d[:, :, 2 * D:])
            nc.sync.dma_start(out=w8l[:, :, 2 * D:], in_=w8l_d[:, :, 2 * D:])
            vbias = singles.tile([128, D], BF)
            nc.sync.dma_start(out=vbias, in_=vb_d[:].to_broadcast([128, D]))
            pbias = singles.tile([128, D], BF)
            nc.sync.dma_start(out=pbias, in_=pb_d[:].to_broadcast([128, D]))
            wp8h = singles.tile([128, KT, D], F8)
            nc.sync.dma_start(out=wp8h, in_=wp8h_d[:])
            wp8l = singles.tile([128, KT, D], F8)
            nc.sync.dma_start(out=wp8l, in_=wp8l_d[:])
            biasT = singles.tile([128, NH, TT, N], BF)
            for h in range(NH):
                nc.scalar.dma_start(out=biasT[:, h], in_=biasT_d[:, h])

            # ---- schedule: 2-head software pipeline + spread fillers ----
            # Prelude of batch b (first 6 qk tiles) runs during batch b-1;
            # its v and remaining qk tiles spread through b's own head loop,
            # together with proj(b-1) and the prelude of b+1.
            units = {0: first_units}
            for i in [0, 2, 4, 1, 3, 5]:       # q0,q1,q2,k0,k1,k2 (dma order)
                first_units["early"][i]()
            for b in range(BL):
                state[b, "o"] = o_pool.tile([128, TT, D], BF, name="o", tag="o")
                fillers = list(units[b]["v"]) + list(units[b]["later"])
                if b > 0:
                    fillers += make_proj_units(b - 1)
                if b + 1 < BL:
                    units[b + 1] = make_qkv_units(b + 1)
                    fillers += units[b + 1]["early"]
                pace = 55
                slot = [0]
                done = [0]

                def fill(need=None):
                    slot[0] += 1
                    due = min(len(fillers), len(fillers) * slot[0] // pace)
                    if need is not None:
                        due = max(due, min(need, len(fillers)))
                    while done[0] < due:
                        fillers[done[0]]()
                        done[0] += 1

                pend = []
                for h in range(NH):
                    pend.append((h, emit_scores(b, h, fill)))
                    if len(pend) > 2:
                        ph, pexp = pend.pop(0)
                        if ph == 0:
                            fill(need=11)   # v + own qk must precede P@V(0)
                        emit_pav(b, ph, pexp)
                    fill()
                for ph, pexp in pend:
                    emit_pav(b, ph, pexp)
                    fill()
                while done[0] < len(fillers):
                    fillers[done[0]]()
                    done[0] += 1
            for u in make_proj_units(BL - 1):
                u()
    nc.finalize()
    return nc


_NC_CACHE = {}


def _get_nc():
    if "nc" not in _NC_CACHE:
        _NC_CACHE["nc"] = _build_nc()
    return _NC_CACHE["nc"]


def _prep_shared(qkv_w, q_bias, v_bias, rpb_table, proj_w, proj_b, rel_index):
    import ml_dtypes
    E4M3 = ml_dtypes.float8_e4m3
    qkv_w = np.asarray(qkv_w, dtype=np.float32)
    # fp8 hi/lo split of SW*W in the [128, KT, 3D] device layout
    wt = np.ascontiguousarray(
        qkv_w.T.reshape(KT, 128, 3 * D).transpose(1, 0, 2)) * SW
    w8h = wt.astype(E4M3)
    w8l = (wt - w8h.astype(np.float32)).astype(E4M3)
    # biases at the SX*SW-scaled level (descale folds into exp / rowsum col)
    qkv_bias = np.concatenate([
        np.asarray(q_bias, np.float32),
        np.zeros(D, np.float32),
        np.asarray(v_bias, np.float32),
    ]) * (SX * SW)
    wpt = np.ascontiguousarray(
        np.asarray(proj_w, np.float32).T.reshape(KT, 128, D)
        .transpose(1, 0, 2)) * SWP
    wp8h = wpt.astype(E4M3)
    wp8l = (wpt - wp8h.astype(np.float32)).astype(E4M3)
    qkvb = np.ascontiguousarray(qkv_bias.reshape(18, 128).T).astype(np.float32)
    # relative position bias, transposed to [k, q] and padded to 640 rows
    rb = np.asarray(rpb_table, np.float32)[
        np.asarray(rel_index).reshape(-1)].reshape(N, N, NH)  # [q, k, h]
    rbp = np.zeros((TT * 128, N, NH), np.float32)
    rbp[:N] = rb.transpose(1, 0, 2)                            # [k, q, h]
    biasT = np.ascontiguousarray(
        np.exp(rbp.reshape(TT, 128, N, NH).transpose(1, 3, 0, 2))).astype(BF16)
    vb = np.ascontiguousarray(qkv_bias[2 * D:].reshape(1, D)).astype(BF16)
    pb = np.ascontiguousarray(
        np.asarray(proj_b, np.float32).reshape(1, D)).astype(BF16)
    return w8h, w8l, wp8h, wp8l, qkvb, biasT, vb, pb


def _make_in_maps(inputs):
    import ml_dtypes
    E4M3 = ml_dtypes.float8_e4m3
    x = np.asarray(inputs["x"], dtype=np.float32)
    w8h, w8l, wp8h, wp8l, qkvb, biasT, vb, pb = _prep_shared(
        inputs["qkv_w"], inputs["q_bias"], inputs["v_bias"],
        inputs["rpb_table"], inputs["proj_w"], inputs["proj_b"],
        inputs["rel_index"])

    in_maps = []
    for i in range(NCORES):
        xs = x[i * BL:(i + 1) * BL]                            # [BL, N, D]
        xT = np.zeros((BL, 128, KT, NP), np.float32)
        xT[..., :N] = xs.transpose(0, 2, 1).reshape(BL, KT, 128, N)\
            .transpose(0, 2, 1, 3) * SX
        x8h = xT.astype(E4M3)
        x8l = (xT - x8h.astype(np.float32)).astype(E4M3)
        in_maps.append({
            "x8h": x8h, "x8l": x8l, "w8h": w8h, "w8l": w8l,
            "wp8h": wp8h, "wp8l": wp8l, "biasT": biasT,
            "qkvb": qkvb, "vb": vb, "pb": pb,
        })

    return in_maps


def kernel(**inputs):
    in_maps = _make_in_maps(inputs)
    nc = _get_nc()
    res = run_bass_kernel_spmd(nc, in_maps, core_ids=list(range(NCORES)))
    out = np.concatenate([res.results[i]["out"] for i in range(NCORES)], axis=0)
    return np.ascontiguousarray(out.astype(np.float32))


def kernel_traced(**inputs):
    """Like kernel() but also returns (out, BassKernelResults with profile)."""
    in_maps = _make_in_maps(inputs)
    nc = _get_nc()
    res = run_bass_kernel_spmd(nc, in_maps, core_ids=list(range(NCORES)),
                               trace=True)
    out = np.concatenate([res.results[i]["out"] for i in range(NCORES)], axis=0)
    return np.ascontiguousarray(out.astype(np.float32)), res

